# revision 1
# baseline (speedup 1.0000x reference)
"""Trainium2 Bass kernel for nn_Attention_9354438771128.

GQA attention block (Mistral-style): QKV projections + RoPE + block-diagonal
(8 x 1024) full attention + output projection, fp32 reference.

Sharding: data-parallel over the 8 sequence blocks, one block per NeuronCore.
Each core computes its block's full attention independently (no collectives).

Per-core pipeline (all matmuls bf16 with fp32 PSUM accumulation):
  - host pre-work: x^T slices, per-head even/odd column permutation of wq/wk
    (turns interleaved RoPE into a half-rotation), RoPE cos/sin tables in
    transposed layout, weight re-layouts for contiguous DMA.
  - q^T/k^T computed per head directly in [head_dim, seq] layout; RoPE applied
    with a partition-swap (SBUF->SBUF DMA) + 3 vector ops.
  - scores^T = k^T.T @ q^T per 128-key tile; exp on ScalarE (fused scale,
    no max subtraction -- scores are bounded ~|9| for this distribution);
    softmax denominator via ones-vector matmul on TensorE; reciprocal on
    VectorE; broadcast via GpSimd partition_broadcast; PV matmul accumulates
    A^T = V^T P^T in PSUM; normalization fused into the PSUM evacuation.
  - A^T staged to DRAM (bf16), then out = A @ wo streamed per column block.
"""

import sys

sys.path.insert(0, "/opt/trn_rl_repo")

import numpy as np
import ml_dtypes

BF = ml_dtypes.bfloat16

B, S, DIM = 8, 1024, 4096
NH, NKV, HD = 32, 8, 128
KC = DIM // 128            # 32 contraction chunks
TT = S // 128              # 8 token tiles per block
SCALE = HD ** -0.5

_CACHE = {}


def _build(repeat=1, phases="BCDE"):
    import concourse.bass as bass
    import concourse.mybir as mybir
    from concourse import bacc, bass_utils
    from concourse.tile import TileContext

    # let walrus elide back-to-back identical weight loads
    if not getattr(bass_utils.get_walrus_args, "_ldw_opt", False):
        _orig = bass_utils.get_walrus_args

        def _patched(*a, **k):
            return [x.replace("--enable-ldw-opt=false", "--enable-ldw-opt=true")
                    for x in _orig(*a, **k)]

        _patched._ldw_opt = True
        bass_utils.get_walrus_args = _patched

    f32 = mybir.dt.float32
    bf16 = mybir.dt.bfloat16
    Exp = mybir.ActivationFunctionType.Exp
    mult = mybir.AluOpType.mult
    add = mybir.AluOpType.add

    nc = bacc.Bacc("TRN2", num_devices=8)

    xT = nc.dram_tensor("xT", [KC, 128, S], bf16, kind="ExternalInput")
    wq = nc.dram_tensor("wq", [NH, 128, DIM], bf16, kind="ExternalInput")
    wk = nc.dram_tensor("wk", [NKV, 128, DIM], bf16, kind="ExternalInput")
    wv = nc.dram_tensor("wv", [KC, 128, NKV * HD], bf16, kind="ExternalInput")
    wo = nc.dram_tensor("wo", [NH, 128, DIM], bf16, kind="ExternalInput")
    cosb = nc.dram_tensor("cosb", [128, S], f32, kind="ExternalInput")
    sinb = nc.dram_tensor("sinb", [128, S], f32, kind="ExternalInput")
    out = nc.dram_tensor("out", [TT, 128, DIM], f32, kind="ExternalOutput")
    at_dram = nc.dram_tensor("at_scratch", [TT, 128, NH * 128], bf16, kind="Internal")

    with TileContext(nc) as tc:
      for _rep in range(repeat):
        with tc.tile_pool(name="const", bufs=1) as cpool:
            cos_t = cpool.tile([128, S], f32)
            sin_t = cpool.tile([128, S], f32)
            ones_t = cpool.tile([128, 1], bf16)
            nc.sync.dma_start(cos_t, cosb[:])
            nc.sync.dma_start(sin_t, sinb[:])
            nc.vector.memset(ones_t, 1.0)

            with tc.tile_pool(name="xt", bufs=1) as xtpool:
                xt_t = xtpool.tile([128, KC, S], bf16)
                for kc in range(KC):
                    nc.sync.dma_start(xt_t[:, kc], xT[kc])

                # ---------------- Phase B/C: K^T (roped) and V ----------------
                with tc.tile_pool(name="kv", bufs=1) as kvpool, \
                     tc.tile_pool(name="wstream", bufs=2) as wpool, \
                     tc.tile_pool(name="rope", bufs=2) as rpool, \
                     tc.tile_pool(name="qkps", bufs=2, space="PSUM") as qkps:
                    kt_t = kvpool.tile([128, NKV, S], bf16)
                    v_t = kvpool.tile([128, TT, NKV * HD], bf16)

                    def rope_store(psum_half, dst, sl):
                        # dst[:, sl] = psum * cos + swap(psum) * sin  (bf16 out)
                        n = psum_half.shape[-1]
                        raw = rpool.tile([128, 512], f32, tag="rope_raw")
                        sw = rpool.tile([128, 512], f32, tag="rope_sw")
                        t1 = rpool.tile([128, 512], f32, tag="rope_t1")
                        t2 = rpool.tile([128, 512], f32, tag="rope_t2")
                        nc.scalar.copy(raw[:, :n], psum_half)
                        nc.scalar.dma_start(sw[0:64, :n], raw[64:128, :n])
                        nc.scalar.dma_start(sw[64:128, :n], raw[0:64, :n])
                        nc.vector.tensor_tensor(t1[:, :n], psum_half, cos_t[:, sl], mult)
                        nc.vector.tensor_tensor(t2[:, :n], sw[:, :n], sin_t[:, sl], mult)
                        nc.vector.tensor_tensor(dst, t1[:, :n], t2[:, :n], add)

                    for g in range(NKV):
                        wk_t = wpool.tile([128, DIM], bf16, tag="wqk")
                        nc.sync.dma_start(wk_t, wk[g])
                        for ch in range(2):
                            sl = slice(ch * 512, (ch + 1) * 512)
                            ps = qkps.tile([128, 512], f32)
                            for kc in range(KC):
                                nc.tensor.matmul(
                                    ps, wk_t[:, kc * 128:(kc + 1) * 128],
                                    xt_t[:, kc, sl],
                                    start=(kc == 0), stop=(kc == KC - 1))
                            rope_store(ps, kt_t[:, g, sl], sl)

                    # V projection: quarters of the 1024 kv columns, paired
                    # so each stationary xT tile feeds two matmuls (hides LDW)
                    with tc.tile_pool(name="wvstream", bufs=2) as wvpool, \
                         tc.tile_pool(name="vps", bufs=2, space="PSUM") as vps:
                        for vp in range(2):
                            wv_a = wvpool.tile([128, KC, 256], bf16, tag="wva")
                            wv_b = wvpool.tile([128, KC, 256], bf16, tag="wvb")
                            nc.sync.dma_start(
                                wv_a, wv[:, :, vp * 512:vp * 512 + 256].rearrange(
                                    "k p n -> p k n"))
                            nc.sync.dma_start(
                                wv_b, wv[:, :, vp * 512 + 256:vp * 512 + 512].rearrange(
                                    "k p n -> p k n"))
                            for tt in range(TT):
                                ps_a = vps.tile([128, 256], f32, tag="vpsa")
                                ps_b = vps.tile([128, 256], f32, tag="vpsb")
                                for kc in range(KC):
                                    lhs = xt_t[:, kc, tt * 128:(tt + 1) * 128]
                                    nc.tensor.matmul(
                                        ps_a, lhs, wv_a[:, kc],
                                        start=(kc == 0), stop=(kc == KC - 1))
                                    nc.tensor.matmul(
                                        ps_b, lhs, wv_b[:, kc],
                                        start=(kc == 0), stop=(kc == KC - 1))
                                nc.vector.tensor_copy(
                                    v_t[:, tt, vp * 512:vp * 512 + 256], ps_a)
                                nc.vector.tensor_copy(
                                    v_t[:, tt, vp * 512 + 256:vp * 512 + 512], ps_b)

                    # ---------------- Phase D: per-head Q + attention ----------------
                    if "D" not in phases:
                        nc.gpsimd.dma_start(out[0, :, :S], kt_t.rearrange("p a b -> p (a b)")[:, :S])
                        nc.gpsimd.dma_start(out[1, :, :S], v_t.rearrange("p a b -> p (a b)")[:, :S])
                        continue
                    with tc.tile_pool(name="qt", bufs=4) as qtpool, \
                         tc.tile_pool(name="expt", bufs=16) as epool, \
                         tc.tile_pool(name="esump", bufs=6) as esump, \
                         tc.tile_pool(name="nrm", bufs=3) as npool, \
                         tc.tile_pool(name="atst", bufs=2) as atpool, \
                         tc.tile_pool(name="sps", bufs=2, space="PSUM") as sps, \
                         tc.tile_pool(name="aps", bufs=2, space="PSUM") as aps, \
                         tc.tile_pool(name="nps", bufs=2, space="PSUM") as nps:
                        for h in range(NH):
                            g = h // 4
                            wq_t = wpool.tile([128, DIM], bf16, tag="wqk")
                            nc.sync.dma_start(wq_t, wq[h])
                            qt_t = qtpool.tile([128, S], bf16)
                            for ch in range(2):
                                sl = slice(ch * 512, (ch + 1) * 512)
                                ps = qkps.tile([128, 512], f32)
                                for kc in range(KC):
                                    nc.tensor.matmul(
                                        ps, wq_t[:, kc * 128:(kc + 1) * 128],
                                        xt_t[:, kc, sl],
                                        start=(kc == 0), stop=(kc == KC - 1))
                                rope_store(ps, qt_t[:, sl], sl)

                            at_t = atpool.tile([128, S], bf16)
                            for ch in range(2):
                                sl = slice(ch * 512, (ch + 1) * 512)
                                a_ps = aps.tile([128, 512], f32)
                                n_ps = nps.tile([1, 512], f32)
                                e_ts = []
                                for sk in range(TT):
                                    s_ps = sps.tile([128, 512], f32)
                                    nc.tensor.matmul(
                                        s_ps,
                                        kt_t[:, g, sk * 128:(sk + 1) * 128],
                                        qt_t[:, sl], start=True, stop=True)
                                    e_t = epool.tile([128, 512], bf16)
                                    nc.scalar.activation(e_t, s_ps, Exp, scale=SCALE)
                                    e_ts.append(e_t)
                                    nc.tensor.matmul(
                                        a_ps,
                                        v_t[:, sk, g * 128:(g + 1) * 128], e_t,
                                        start=(sk == 0), stop=(sk == TT - 1))
                                # partial softmax denominator: elementwise tree
                                # over the 8 key tiles on DVE, then one 128-row
                                # reduction matmul with the ones vector.
                                lvl = e_ts
                                while len(lvl) > 1:
                                    nxt = []
                                    for i in range(0, len(lvl), 2):
                                        s_t = esump.tile([128, 512], bf16, tag="esum")
                                        nc.vector.tensor_tensor(
                                            s_t, lvl[i], lvl[i + 1], add)
                                        nxt.append(s_t)
                                    lvl = nxt
                                nc.tensor.matmul(n_ps, ones_t, lvl[0],
                                                 start=True, stop=True)
                                rec_t = npool.tile([1, 512], f32, tag="rec")
                                nc.vector.reciprocal(rec_t, n_ps)
                                nb_t = npool.tile([128, 512], f32, tag="nb")
                                nc.gpsimd.partition_broadcast(nb_t, rec_t)
                                nc.vector.tensor_tensor(at_t[:, sl], a_ps, nb_t, mult)
                            nc.scalar.dma_start(
                                at_dram[:, :, h * 128:(h + 1) * 128].rearrange(
                                    "a p m -> p a m"),
                                at_t.rearrange("p (a m) -> p a m", a=TT))

        # ---------------- Phase E: out = A @ wo ----------------
        if "E" not in phases:
            continue
        with tc.tile_pool(name="wo", bufs=2) as wopool, \
             tc.tile_pool(name="atrd", bufs=1) as atrd, \
             tc.tile_pool(name="ost", bufs=4) as opool, \
             tc.tile_pool(name="ops", bufs=4, space="PSUM") as ops:
            at_all = atrd.tile([128, TT, NH * 128], bf16)
            for nch in range(8):
                nsl = slice(nch * 512, (nch + 1) * 512)
                wo_t = wopool.tile([128, NH, 512], bf16)
                nc.sync.dma_start(wo_t, wo[:, :, nsl].rearrange("h p n -> p h n"))
                if nch == 0:
                    for tt in range(TT):
                        nc.sync.dma_start(at_all[:, tt], at_dram[tt])
                for tt in range(TT):
                    o_ps = ops.tile([128, 512], f32)
                    for h in range(NH):
                        nc.tensor.matmul(
                            o_ps, at_all[:, tt, h * 128:(h + 1) * 128], wo_t[:, h],
                            start=(h == 0), stop=(h == NH - 1))
                    o_t = opool.tile([128, 512], f32)
                    nc.scalar.copy(o_t, o_ps)
                    nc.sync.dma_start(out[tt, :, nsl], o_t)

    nc.compile()
    return nc


def _prep_shared(wq, wk, wv, wo):
    idx = np.arange(128)
    ph = np.concatenate([idx[0::2], idx[1::2]])
    permq = (np.arange(NH)[:, None] * HD + ph[None, :]).reshape(-1)
    permk = (np.arange(NKV)[:, None] * HD + ph[None, :]).reshape(-1)
    wq_r = np.ascontiguousarray(
        wq[:, permq].reshape(KC, 128, NH, HD).transpose(2, 1, 0, 3)
    ).reshape(NH, 128, DIM).astype(BF)
    wk_r = np.ascontiguousarray(
        wk[:, permk].reshape(KC, 128, NKV, HD).transpose(2, 1, 0, 3)
    ).reshape(NKV, 128, DIM).astype(BF)
    wv_r = wv.reshape(KC, 128, NKV * HD).astype(BF)
    wo_r = wo.reshape(NH, 128, DIM).astype(BF)
    return wq_r, wk_r, wv_r, wo_r


def make_in_maps(x, freqs_cos, freqs_sin, wq, wk, wv, wo):
    wq_r, wk_r, wv_r, wo_r = _prep_shared(
        np.asarray(wq, np.float32), np.asarray(wk, np.float32),
        np.asarray(wv, np.float32), np.asarray(wo, np.float32))

    x = np.asarray(x, np.float32)
    fc = np.asarray(freqs_cos, np.float32)
    fs = np.asarray(freqs_sin, np.float32)

    in_maps = []
    for b in range(B):
        xb = x[b * S:(b + 1) * S]                       # [S, DIM]
        xT_b = np.ascontiguousarray(xb.T).astype(BF).reshape(KC, 128, S)
        c = np.ascontiguousarray(fc[b * S:(b + 1) * S].T.astype(np.float32))
        s = np.ascontiguousarray(fs[b * S:(b + 1) * S].T.astype(np.float32))
        cosb = np.concatenate([c, c], axis=0)           # [128, S]
        sinb = np.concatenate([-s, s], axis=0)
        in_maps.append({
            "xT": xT_b, "wq": wq_r, "wk": wk_r, "wv": wv_r, "wo": wo_r,
            "cosb": np.ascontiguousarray(cosb),
            "sinb": np.ascontiguousarray(sinb),
        })
    return in_maps


def kernel(x, freqs_cos, freqs_sin, wq, wk, wv, wo):
    from concourse.bass_utils import run_bass_kernel_spmd

    if "nc" not in _CACHE:
        _CACHE["nc"] = _build()
    nc = _CACHE["nc"]

    in_maps = make_in_maps(x, freqs_cos, freqs_sin, wq, wk, wv, wo)

    res = run_bass_kernel_spmd(nc, in_maps, core_ids=list(range(B)))
    _CACHE["last_results"] = res
    outs = [r["out"].reshape(S, DIM) for r in res.results]
    return np.concatenate(outs, axis=0)



# revision 14
# speedup vs baseline: 60.2885x; 60.2885x over previous
"""Trainium2 Bass kernel for nn_Attention_9354438771128.

GQA attention block (Mistral-style): QKV projections + RoPE + block-diagonal
(8 x 1024) full attention + output projection, fp32 reference.

Sharding: data-parallel over the 8 sequence blocks, one block per NeuronCore.
Each core computes its block's full attention independently (no collectives).

Per-core pipeline (all matmuls bf16 with fp32 PSUM accumulation):
  - host pre-work: x^T slices, per-head even/odd column permutation of wq/wk
    (turns interleaved RoPE into a half-rotation), RoPE cos/sin tables in
    transposed layout, wv/wo pre-tiled so every weight DMA is one fully
    contiguous block.
  - k^T/q^T computed per head directly in [head_dim, seq] layout; RoPE applied
    with 4 DVE ops using partition-offset operands (no SBUF->SBUF swap DMA).
  - per head: scores^T = k^T.T @ q^T into a 2-bank [128,1024] PSUM tile
    (LDW shared between the two 512-wide matmuls); exp on ScalarE (fused
    scale, no max subtraction -- scores bounded ~|9|); running softmax
    denominator on DVE; PV accumulates A^T = V^T P^T in PSUM; denominator
    all-reduced+broadcast across partitions by GpSimd partition_all_reduce;
    normalization fused into the PSUM evacuation; A^T staged to DRAM (bf16,
    one contiguous store per head).
  - out = A @ wo streamed per 512-column block, one contiguous DMA per block.
"""

import sys

sys.path.insert(0, "/opt/trn_rl_repo")

import numpy as np
import ml_dtypes

BF = ml_dtypes.bfloat16

B, S, DIM = 8, 1024, 4096
NH, NKV, HD = 32, 8, 128
KC = DIM // 128            # 32 contraction chunks
TT = S // 128              # 8 token tiles per block
SCALE = HD ** -0.5

_CACHE = {}


def _build(repeat=1, phases="BDE", loop=0):
    import concourse.bass as bass
    import concourse.mybir as mybir
    from concourse import bacc, bass_utils
    from concourse.tile import TileContext
    from bass_rust import ReduceOp

    # let walrus elide back-to-back identical weight loads
    if not getattr(bass_utils.get_walrus_args, "_ldw_opt", False):
        _orig = bass_utils.get_walrus_args

        def _patched(*a, **k):
            return [x.replace("--enable-ldw-opt=false", "--enable-ldw-opt=true")
                    for x in _orig(*a, **k)]

        _patched._ldw_opt = True
        bass_utils.get_walrus_args = _patched

    f32 = mybir.dt.float32
    bf16 = mybir.dt.bfloat16
    Exp = mybir.ActivationFunctionType.Exp
    mult = mybir.AluOpType.mult
    add = mybir.AluOpType.add

    nc = bacc.Bacc("TRN2", num_devices=8)

    xT = nc.dram_tensor("xT", [KC, 128, S], bf16, kind="ExternalInput")
    wq = nc.dram_tensor("wq", [NH, 128, DIM], bf16, kind="ExternalInput")
    wk = nc.dram_tensor("wk", [NKV, 128, DIM], bf16, kind="ExternalInput")
    wv = nc.dram_tensor("wv", [2, 128, KC, 512], bf16, kind="ExternalInput")
    wo = nc.dram_tensor("wo", [8, 128, NH, 512], bf16, kind="ExternalInput")
    cosb = nc.dram_tensor("cosb", [128, S], f32, kind="ExternalInput")
    sinb = nc.dram_tensor("sinb", [128, S], f32, kind="ExternalInput")
    out = nc.dram_tensor("out", [TT, 128, DIM], f32, kind="ExternalOutput")
    at_dram = nc.dram_tensor("at_scratch", [NH, 128, S], bf16, kind="Internal")

    def body(tc):
        with tc.tile_pool(name="const", bufs=1) as cpool:
            cos_t = cpool.tile([128, S], f32)
            sin_t = cpool.tile([128, S], f32)
            nc.sync.dma_start(cos_t, cosb[:])
            nc.sync.dma_start(sin_t, sinb[:])

            def rope_store(psum_half, dst, sl):
                # dst = psum * cos + halfswap(psum) * sin  (sign folded into
                # the host-built sin table); partition-offset DVE operands
                # replace the SBUF->SBUF partition-swap DMA.
                t1 = rpool.tile([128, 512], f32, tag="rope_t1")
                t2 = rpool.tile([128, 512], f32, tag="rope_t2")
                nc.vector.tensor_tensor(t1, psum_half, cos_t[:, sl], mult)
                nc.vector.tensor_tensor(
                    t2[0:64], psum_half[64:128], sin_t[0:64, sl], mult)
                nc.vector.tensor_tensor(
                    t2[64:128], psum_half[0:64], sin_t[64:128, sl], mult)
                nc.vector.tensor_tensor(dst, t1, t2, add)

            with tc.tile_pool(name="xt", bufs=1) as xtpool, \
                 tc.tile_pool(name="wstream", bufs=3) as wpool, \
                 tc.tile_pool(name="rope", bufs=2) as rpool, \
                 tc.tile_pool(name="kv", bufs=1) as kvpool, \
                 tc.tile_pool(name="qkps", bufs=2, space="PSUM") as qkps:
                xt_t = xtpool.tile([128, KC, S], bf16)
                for kc in range(KC):
                    nc.sync.dma_start(xt_t[:, kc], xT[kc])

                kt_t = kvpool.tile([128, NKV, S], bf16)
                v_t = kvpool.tile([128, TT, NKV * HD], bf16)

                # ---------------- Phase B: K^T (roped) and V ----------------
                for g in range(NKV):
                    wk_t = wpool.tile([128, DIM], bf16, tag="wqk")
                    nc.sync.dma_start(wk_t, wk[g])
                    for ch in range(2):
                        sl = slice(ch * 512, (ch + 1) * 512)
                        ps = qkps.tile([128, 512], f32, tag="qk")
                        for kc in range(KC):
                            nc.tensor.matmul(
                                ps, wk_t[:, kc * 128:(kc + 1) * 128],
                                xt_t[:, kc, sl],
                                start=(kc == 0), stop=(kc == KC - 1))
                        rope_store(ps, kt_t[:, g, sl], sl)

                with tc.tile_pool(name="wvstream", bufs=2) as wvpool, \
                     tc.tile_pool(name="vps", bufs=2, space="PSUM") as vps:
                    for vc in range(2):
                        wv_t = wvpool.tile([128, KC, 512], bf16)
                        nc.sync.dma_start(wv_t, wv[vc])
                        for tt in range(TT):
                            ps = vps.tile([128, 512], f32)
                            for kc in range(KC):
                                nc.tensor.matmul(
                                    ps, xt_t[:, kc, tt * 128:(tt + 1) * 128],
                                    wv_t[:, kc],
                                    start=(kc == 0), stop=(kc == KC - 1))
                            nc.vector.tensor_copy(
                                v_t[:, tt, vc * 512:(vc + 1) * 512], ps)

                # ---------------- Phase D: per-head Q + attention ----------------
                if "D" not in phases:
                    nc.gpsimd.dma_start(
                        out[0, :, :S],
                        kt_t.rearrange("p a b -> p (a b)")[:, :S])
                    nc.gpsimd.dma_start(
                        out[1, :, :S],
                        v_t.rearrange("p a b -> p (a b)")[:, :S])
                    return
                with tc.tile_pool(name="qt", bufs=3) as qtpool, \
                     tc.tile_pool(name="expt", bufs=4) as epool, \
                     tc.tile_pool(name="esum", bufs=4) as espool, \
                     tc.tile_pool(name="nrm", bufs=2) as npool, \
                     tc.tile_pool(name="atst", bufs=2) as atpool, \
                     tc.tile_pool(name="sps", bufs=2, space="PSUM") as sps, \
                     tc.tile_pool(name="aps", bufs=1, space="PSUM") as aps:
                    for h in range(NH):
                        g = h // 4
                        wq_t = wpool.tile([128, DIM], bf16, tag="wqk")
                        nc.sync.dma_start(wq_t, wq[h])
                        qt_t = qtpool.tile([128, S], bf16)
                        for ch in range(2):
                            sl = slice(ch * 512, (ch + 1) * 512)
                            ps = qkps.tile([128, 512], f32, tag="qk")
                            for kc in range(KC):
                                nc.tensor.matmul(
                                    ps, wq_t[:, kc * 128:(kc + 1) * 128],
                                    xt_t[:, kc, sl],
                                    start=(kc == 0), stop=(kc == KC - 1))
                            rope_store(ps, qt_t[:, sl], sl)

                        a_ps = aps.tile([128, S], f32)
                        es_prev = None
                        for sk in range(TT):
                            kt_sl = kt_t[:, g, sk * 128:(sk + 1) * 128]
                            s_ps = sps.tile([128, S], f32)
                            nc.tensor.matmul(
                                s_ps[:, 0:512], kt_sl, qt_t[:, 0:512],
                                start=True, stop=True)
                            nc.tensor.matmul(
                                s_ps[:, 512:1024], kt_sl, qt_t[:, 512:1024],
                                start=True, stop=True)
                            e_t = epool.tile([128, S], bf16)
                            nc.scalar.activation(e_t, s_ps, Exp, scale=SCALE)
                            if es_prev is None:
                                es_prev = e_t
                            else:
                                es_new = espool.tile([128, S], bf16, tag="es")
                                nc.vector.tensor_tensor(es_new, es_prev, e_t, add)
                                es_prev = es_new
                            v_sl = v_t[:, sk, g * 128:(g + 1) * 128]
                            nc.tensor.matmul(
                                a_ps[:, 0:512], v_sl, e_t[:, 0:512],
                                start=(sk == 0), stop=(sk == TT - 1))
                            nc.tensor.matmul(
                                a_ps[:, 512:1024], v_sl, e_t[:, 512:1024],
                                start=(sk == 0), stop=(sk == TT - 1))

                        nb_t = npool.tile([128, S], f32, tag="nb")
                        nc.gpsimd.partition_all_reduce(
                            nb_t, es_prev, 128, ReduceOp.add)
                        rec_t = npool.tile([128, S], f32, tag="rec")
                        nc.vector.reciprocal(rec_t, nb_t)
                        at_t = atpool.tile([128, S], bf16)
                        nc.vector.tensor_tensor(at_t, a_ps, rec_t, mult)
                        nc.sync.dma_start(at_dram[h], at_t)

        # ---------------- Phase E: out = A @ wo ----------------
        if "E" not in phases:
            return
        with tc.tile_pool(name="atrd", bufs=1) as atrd, \
             tc.tile_pool(name="wo", bufs=3) as wopool, \
             tc.tile_pool(name="ost", bufs=6) as opool, \
             tc.tile_pool(name="ops", bufs=4, space="PSUM") as ops:
            at_all = atrd.tile([128, NH, S], bf16)
            for h in range(NH):
                nc.sync.dma_start(at_all[:, h], at_dram[h])
            for nch in range(8):
                nsl = slice(nch * 512, (nch + 1) * 512)
                wo_t = wopool.tile([128, NH, 512], bf16, tag="wo_t")
                nc.sync.dma_start(wo_t, wo[nch])
                for tt in range(TT):
                    o_ps = ops.tile([128, 512], f32)
                    for h in range(NH):
                        nc.tensor.matmul(
                            o_ps, at_all[:, h, tt * 128:(tt + 1) * 128],
                            wo_t[:, h],
                            start=(h == 0), stop=(h == NH - 1))
                    o_t = opool.tile([128, 512], f32)
                    nc.vector.tensor_copy(o_t, o_ps)
                    nc.sync.dma_start(out[tt, :, nsl], o_t)

    with TileContext(nc) as tc:
        if loop > 1:
            with tc.For_i(0, loop, 1):
                body(tc)
        else:
            for _rep in range(repeat):
                body(tc)

    nc.compile()
    return nc


def _prep_shared(wq, wk, wv, wo):
    idx = np.arange(128)
    ph = np.concatenate([idx[0::2], idx[1::2]])
    permq = (np.arange(NH)[:, None] * HD + ph[None, :]).reshape(-1)
    permk = (np.arange(NKV)[:, None] * HD + ph[None, :]).reshape(-1)
    wq_r = np.ascontiguousarray(
        wq[:, permq].reshape(KC, 128, NH, HD).transpose(2, 1, 0, 3)
    ).reshape(NH, 128, DIM).astype(BF)
    wk_r = np.ascontiguousarray(
        wk[:, permk].reshape(KC, 128, NKV, HD).transpose(2, 1, 0, 3)
    ).reshape(NKV, 128, DIM).astype(BF)
    wv_r = np.ascontiguousarray(
        wv.reshape(KC, 128, 2, 512).transpose(2, 1, 0, 3)).astype(BF)
    wo_r = np.ascontiguousarray(
        wo.reshape(NH, 128, 8, 512).transpose(2, 1, 0, 3)).astype(BF)
    return wq_r, wk_r, wv_r, wo_r


def make_in_maps(x, freqs_cos, freqs_sin, wq, wk, wv, wo):
    wq_r, wk_r, wv_r, wo_r = _prep_shared(
        np.asarray(wq, np.float32), np.asarray(wk, np.float32),
        np.asarray(wv, np.float32), np.asarray(wo, np.float32))

    x = np.asarray(x, np.float32)
    fc = np.asarray(freqs_cos, np.float32)
    fs = np.asarray(freqs_sin, np.float32)

    in_maps = []
    for b in range(B):
        xb = x[b * S:(b + 1) * S]                       # [S, DIM]
        xT_b = np.ascontiguousarray(xb.T).astype(BF).reshape(KC, 128, S)
        c = np.ascontiguousarray(fc[b * S:(b + 1) * S].T.astype(np.float32))
        s = np.ascontiguousarray(fs[b * S:(b + 1) * S].T.astype(np.float32))
        cosb = np.concatenate([c, c], axis=0)           # [128, S]
        sinb = np.concatenate([-s, s], axis=0)
        in_maps.append({
            "xT": xT_b, "wq": wq_r, "wk": wk_r, "wv": wv_r, "wo": wo_r,
            "cosb": np.ascontiguousarray(cosb),
            "sinb": np.ascontiguousarray(sinb),
        })
    return in_maps


def kernel(x, freqs_cos, freqs_sin, wq, wk, wv, wo):
    from concourse.bass_utils import run_bass_kernel_spmd

    if "nc" not in _CACHE:
        _CACHE["nc"] = _build()
    nc = _CACHE["nc"]

    in_maps = make_in_maps(x, freqs_cos, freqs_sin, wq, wk, wv, wo)

    res = run_bass_kernel_spmd(nc, in_maps, core_ids=list(range(B)))
    _CACHE["last_results"] = res
    outs = [r["out"].reshape(S, DIM) for r in res.results]
    return np.concatenate(outs, axis=0)


# revision 23
# speedup vs baseline: 63.9838x; 1.0613x over previous
"""Trainium2 Bass kernel for nn_Attention_9354438771128.

GQA attention block (Mistral-style): QKV projections + RoPE + block-diagonal
(8 x 1024) full attention + output projection, fp32 reference.

Sharding: data-parallel over the 8 sequence blocks, one block per NeuronCore.
Each core computes its block's full attention independently (no collectives).

Per-core pipeline (all matmuls bf16 with fp32 PSUM accumulation):
  - host pre-work: x^T slices, per-head even/odd column permutation of wq/wk
    (turns interleaved RoPE into a half-rotation), RoPE cos/sin tables in
    transposed layout, wv/wo pre-tiled so every weight DMA is one fully
    contiguous block.
  - k^T/q^T computed per head directly in [head_dim, seq] layout; RoPE applied
    with 4 DVE ops using partition-offset operands (no SBUF->SBUF swap DMA).
  - per head: scores^T = k^T.T @ q^T into a 2-bank [128,1024] PSUM tile
    (LDW shared between the two 512-wide matmuls); exp on ScalarE (fused
    scale, no max subtraction -- scores bounded ~|9|); running softmax
    denominator on DVE; PV accumulates A^T = V^T P^T in PSUM; denominator
    all-reduced+broadcast across partitions by GpSimd partition_all_reduce;
    normalization fused into the PSUM evacuation; A^T staged to DRAM (bf16,
    one contiguous store per head).
  - out = A @ wo streamed per 512-column block, one contiguous DMA per block.
"""

import sys

sys.path.insert(0, "/opt/trn_rl_repo")

import numpy as np
import ml_dtypes

BF = ml_dtypes.bfloat16

B, S, DIM = 8, 1024, 4096
NH, NKV, HD = 32, 8, 128
KC = DIM // 128            # 32 contraction chunks
TT = S // 128              # 8 token tiles per block
SCALE = HD ** -0.5

_CACHE = {}


def _build(repeat=1, phases="BDE", loop=0, pipeline_heads=True,
           early_evac=False, psum_alt=False, nogp=False):
    import concourse.bass as bass
    import concourse.mybir as mybir
    from concourse import bacc, bass_utils
    from concourse.tile import TileContext
    from bass_rust import ReduceOp

    # let walrus elide back-to-back identical weight loads
    if not getattr(bass_utils.get_walrus_args, "_ldw_opt", False):
        _orig = bass_utils.get_walrus_args

        def _patched(*a, **k):
            return [x.replace("--enable-ldw-opt=false", "--enable-ldw-opt=true")
                    for x in _orig(*a, **k)]

        _patched._ldw_opt = True
        bass_utils.get_walrus_args = _patched

    f32 = mybir.dt.float32
    bf16 = mybir.dt.bfloat16
    Exp = mybir.ActivationFunctionType.Exp
    mult = mybir.AluOpType.mult
    add = mybir.AluOpType.add

    nc = bacc.Bacc("TRN2", num_devices=8)

    xT = nc.dram_tensor("xT", [KC, 128, S], bf16, kind="ExternalInput")
    wq = nc.dram_tensor("wq", [NH, 128, DIM], bf16, kind="ExternalInput")
    wk = nc.dram_tensor("wk", [NKV, 128, DIM], bf16, kind="ExternalInput")
    wv = nc.dram_tensor("wv", [2, 128, KC, 512], bf16, kind="ExternalInput")
    wo = nc.dram_tensor("wo", [8, 128, NH, 512], bf16, kind="ExternalInput")
    cosb = nc.dram_tensor("cosb", [128, S], f32, kind="ExternalInput")
    sinb = nc.dram_tensor("sinb", [128, S], f32, kind="ExternalInput")
    out = nc.dram_tensor("out", [TT, 128, DIM], f32, kind="ExternalOutput")
    at_dram = nc.dram_tensor("at_scratch", [NH, 128, S], bf16, kind="Internal")

    def body(tc):
        with tc.tile_pool(name="const", bufs=1) as cpool:
            cos_t = cpool.tile([128, S], f32)
            sin_t = cpool.tile([128, S], f32)
            nc.sync.dma_start(cos_t, cosb[:])
            nc.sync.dma_start(sin_t, sinb[:])
            if nogp:
                ones_col = cpool.tile([128, 1], bf16)
                ones_row = cpool.tile([1, 128], bf16)
                nc.vector.memset(ones_col, 1.0)
                nc.vector.memset(ones_row, 1.0)

            def rope_store(psum_half, dst, sl):
                # dst = psum * cos + halfswap(psum) * sin  (sign folded into
                # the host-built sin table); partition-offset DVE operands
                # replace the SBUF->SBUF partition-swap DMA.
                t1 = rpool.tile([128, 512], f32, tag="rope_t1")
                t2 = rpool.tile([128, 512], f32, tag="rope_t2")
                nc.vector.tensor_tensor(t1, psum_half, cos_t[:, sl], mult)
                nc.vector.tensor_tensor(
                    t2[0:64], psum_half[64:128], sin_t[0:64, sl], mult)
                nc.vector.tensor_tensor(
                    t2[64:128], psum_half[0:64], sin_t[64:128, sl], mult)
                nc.vector.tensor_tensor(dst, t1, t2, add)

            with tc.tile_pool(name="xt", bufs=1) as xtpool, \
                 tc.tile_pool(name="wstream", bufs=3) as wpool, \
                 tc.tile_pool(name="rope", bufs=2) as rpool, \
                 tc.tile_pool(name="kv", bufs=1) as kvpool, \
                 tc.tile_pool(name="qkps", bufs=2, space="PSUM") as qkps:
                xt_t = xtpool.tile([128, KC, S], bf16)
                for kc in range(KC):
                    nc.sync.dma_start(xt_t[:, kc], xT[kc])

                kt_t = kvpool.tile([128, NKV, S], bf16)
                v_t = kvpool.tile([128, TT, NKV * HD], bf16)

                # ---------------- Phase B: K^T (roped) and V ----------------
                for g in range(NKV):
                    wk_t = wpool.tile([128, DIM], bf16, tag="wqk")
                    nc.sync.dma_start(wk_t, wk[g])
                    for ch in range(2):
                        sl = slice(ch * 512, (ch + 1) * 512)
                        ps = qkps.tile([128, 512], f32, tag="qk")
                        for kc in range(KC):
                            nc.tensor.matmul(
                                ps, wk_t[:, kc * 128:(kc + 1) * 128],
                                xt_t[:, kc, sl],
                                start=(kc == 0), stop=(kc == KC - 1))
                        rope_store(ps, kt_t[:, g, sl], sl)

                with tc.tile_pool(name="wvstream", bufs=2) as wvpool, \
                     tc.tile_pool(name="vps", bufs=2, space="PSUM") as vps:
                    for vc in range(2):
                        wv_t = wvpool.tile([128, KC, 512], bf16)
                        nc.sync.dma_start(wv_t, wv[vc])
                        for tt in range(TT):
                            ps = vps.tile([128, 512], f32)
                            for kc in range(KC):
                                nc.tensor.matmul(
                                    ps, xt_t[:, kc, tt * 128:(tt + 1) * 128],
                                    wv_t[:, kc],
                                    start=(kc == 0), stop=(kc == KC - 1))
                            nc.vector.tensor_copy(
                                v_t[:, tt, vc * 512:(vc + 1) * 512], ps)

                # ---------------- Phase D: per-head Q + attention ----------------
                if "D" not in phases:
                    nc.gpsimd.dma_start(
                        out[0, :, :S],
                        kt_t.rearrange("p a b -> p (a b)")[:, :S])
                    nc.gpsimd.dma_start(
                        out[1, :, :S],
                        v_t.rearrange("p a b -> p (a b)")[:, :S])
                    return
                with tc.tile_pool(name="qt", bufs=3) as qtpool, \
                     tc.tile_pool(name="expt", bufs=4) as epool, \
                     tc.tile_pool(name="esum", bufs=4) as espool, \
                     tc.tile_pool(name="nrm", bufs=2) as npool, \
                     tc.tile_pool(name="atst", bufs=2) as atpool, \
                     tc.tile_pool(name="sps", bufs=(1 if psum_alt else 2),
                                  space="PSUM") as sps, \
                     tc.tile_pool(name="aps", bufs=(2 if psum_alt else 1),
                                  space="PSUM") as aps:

                    def qproj(h, qt_t):
                        wq_t = wpool.tile([128, DIM], bf16, tag="wqk")
                        nc.sync.dma_start(wq_t, wq[h])
                        for ch in range(2):
                            sl = slice(ch * 512, (ch + 1) * 512)
                            ps = qkps.tile([128, 512], f32, tag="qk")
                            for kc in range(KC):
                                nc.tensor.matmul(
                                    ps, wq_t[:, kc * 128:(kc + 1) * 128],
                                    xt_t[:, kc, sl],
                                    start=(kc == 0), stop=(kc == KC - 1))
                            rope_store(ps, qt_t[:, sl], sl)

                    qts = {}
                    if pipeline_heads:
                        qts[0] = qtpool.tile([128, S], bf16, tag="qt_t", name="qt_t")
                        qproj(0, qts[0])
                    for h in range(NH):
                        g = h // 4
                        if pipeline_heads:
                            if h + 1 < NH:
                                qts[h + 1] = qtpool.tile(
                                    [128, S], bf16, tag="qt_t", name="qt_t")
                                qproj(h + 1, qts[h + 1])
                            qt_t = qts.pop(h)
                        else:
                            qt_t = qtpool.tile([128, S], bf16, tag="qt_t")
                            qproj(h, qt_t)

                        a_ps = aps.tile([128, S], f32)
                        es_prev = None
                        for sk in range(TT):
                            kt_sl = kt_t[:, g, sk * 128:(sk + 1) * 128]
                            s_ps = sps.tile([128, S], f32)
                            nc.tensor.matmul(
                                s_ps[:, 0:512], kt_sl, qt_t[:, 0:512],
                                start=True, stop=True)
                            nc.tensor.matmul(
                                s_ps[:, 512:1024], kt_sl, qt_t[:, 512:1024],
                                start=True, stop=True)
                            e_t = epool.tile([128, S], bf16)
                            nc.scalar.activation(e_t, s_ps, Exp, scale=SCALE)
                            if es_prev is None:
                                es_prev = e_t
                            else:
                                es_new = espool.tile([128, S], bf16, tag="es")
                                nc.vector.tensor_tensor(es_new, es_prev, e_t, add)
                                es_prev = es_new
                            v_sl = v_t[:, sk, g * 128:(g + 1) * 128]
                            nc.tensor.matmul(
                                a_ps[:, 0:512], v_sl, e_t[:, 0:512],
                                start=(sk == 0), stop=(sk == TT - 1))
                            nc.tensor.matmul(
                                a_ps[:, 512:1024], v_sl, e_t[:, 512:1024],
                                start=(sk == 0), stop=(sk == TT - 1))

                        if nogp:
                            # denominator without GpSimd (its SBUF port lock
                            # stalls DVE): PE ones-matmul partition reduction,
                            # DVE reciprocal, PE outer-product broadcast,
                            # ScalarE copy to SBUF for the DVE normalize.
                            n_ps0 = qkps.tile([1, 512], f32, tag="qk",
                                              name="n_ps0")
                            n_ps1 = qkps.tile([1, 512], f32, tag="qk",
                                              name="n_ps1")
                            nc.tensor.matmul(n_ps0, ones_col,
                                             es_prev[:, 0:512],
                                             start=True, stop=True)
                            nc.tensor.matmul(n_ps1, ones_col,
                                             es_prev[:, 512:1024],
                                             start=True, stop=True)
                            rec1_t = npool.tile([1, S], bf16, tag="rec1")
                            with nc.allow_low_precision(
                                    reason="bf16 reciprocal feeds bf16 matmul"):
                                nc.vector.reciprocal(rec1_t[:, 0:512], n_ps0)
                                nc.vector.reciprocal(
                                    rec1_t[:, 512:1024], n_ps1)
                            nb_ps = sps.tile([128, S], f32, tag="s_ps",
                                             name="nb_ps")
                            nc.tensor.matmul(nb_ps[:, 0:512], ones_row,
                                             rec1_t[:, 0:512],
                                             start=True, stop=True)
                            nc.tensor.matmul(nb_ps[:, 512:1024], ones_row,
                                             rec1_t[:, 512:1024],
                                             start=True, stop=True)
                            rec_t = npool.tile([128, S], f32, tag="nb")
                            nc.scalar.copy(rec_t, nb_ps)
                        else:
                            nb_t = npool.tile([128, S], f32, tag="nb")
                            nc.gpsimd.partition_all_reduce(
                                nb_t, es_prev, 128, ReduceOp.add)
                            rec_t = npool.tile([128, S], f32, tag="rec")
                            nc.vector.reciprocal(rec_t, nb_t)
                        at_t = atpool.tile([128, S], bf16)
                        if early_evac:
                            # free the PSUM bank before the gpsimd/recip chain
                            # completes so the next head's PV is not gated on it
                            ar_t = npool.tile([128, S], f32, tag="ar")
                            nc.vector.tensor_copy(ar_t, a_ps)
                            nc.vector.tensor_tensor(at_t, ar_t, rec_t, mult)
                        else:
                            nc.vector.tensor_tensor(at_t, a_ps, rec_t, mult)
                        nc.sync.dma_start(at_dram[h], at_t)

        # ---------------- Phase E: out = A @ wo ----------------
        if "E" not in phases:
            return
        with tc.tile_pool(name="atrd", bufs=1) as atrd, \
             tc.tile_pool(name="wo", bufs=3) as wopool, \
             tc.tile_pool(name="ost", bufs=6) as opool, \
             tc.tile_pool(name="ops", bufs=4, space="PSUM") as ops:
            at_all = atrd.tile([128, NH, S], bf16)
            for h in range(NH):
                nc.sync.dma_start(at_all[:, h], at_dram[h])
            for nch in range(8):
                nsl = slice(nch * 512, (nch + 1) * 512)
                wo_t = wopool.tile([128, NH, 512], bf16, tag="wo_t")
                nc.sync.dma_start(wo_t, wo[nch])
                for tt in range(TT):
                    o_ps = ops.tile([128, 512], f32)
                    for h in range(NH):
                        nc.tensor.matmul(
                            o_ps, at_all[:, h, tt * 128:(tt + 1) * 128],
                            wo_t[:, h],
                            start=(h == 0), stop=(h == NH - 1))
                    o_t = opool.tile([128, 512], f32)
                    nc.vector.tensor_copy(o_t, o_ps)
                    nc.sync.dma_start(out[tt, :, nsl], o_t)

    with TileContext(nc) as tc:
        if loop > 1:
            with tc.For_i(0, loop, 1):
                body(tc)
        else:
            for _rep in range(repeat):
                body(tc)

    nc.compile()
    return nc


def _prep_shared(wq, wk, wv, wo):
    idx = np.arange(128)
    ph = np.concatenate([idx[0::2], idx[1::2]])
    permq = (np.arange(NH)[:, None] * HD + ph[None, :]).reshape(-1)
    permk = (np.arange(NKV)[:, None] * HD + ph[None, :]).reshape(-1)
    wq_r = np.ascontiguousarray(
        wq[:, permq].reshape(KC, 128, NH, HD).transpose(2, 1, 0, 3)
    ).reshape(NH, 128, DIM).astype(BF)
    wk_r = np.ascontiguousarray(
        wk[:, permk].reshape(KC, 128, NKV, HD).transpose(2, 1, 0, 3)
    ).reshape(NKV, 128, DIM).astype(BF)
    wv_r = np.ascontiguousarray(
        wv.reshape(KC, 128, 2, 512).transpose(2, 1, 0, 3)).astype(BF)
    wo_r = np.ascontiguousarray(
        wo.reshape(NH, 128, 8, 512).transpose(2, 1, 0, 3)).astype(BF)
    return wq_r, wk_r, wv_r, wo_r


def make_in_maps(x, freqs_cos, freqs_sin, wq, wk, wv, wo):
    wq_r, wk_r, wv_r, wo_r = _prep_shared(
        np.asarray(wq, np.float32), np.asarray(wk, np.float32),
        np.asarray(wv, np.float32), np.asarray(wo, np.float32))

    x = np.asarray(x, np.float32)
    fc = np.asarray(freqs_cos, np.float32)
    fs = np.asarray(freqs_sin, np.float32)

    in_maps = []
    for b in range(B):
        xb = x[b * S:(b + 1) * S]                       # [S, DIM]
        xT_b = np.ascontiguousarray(xb.T).astype(BF).reshape(KC, 128, S)
        c = np.ascontiguousarray(fc[b * S:(b + 1) * S].T.astype(np.float32))
        s = np.ascontiguousarray(fs[b * S:(b + 1) * S].T.astype(np.float32))
        cosb = np.concatenate([c, c], axis=0)           # [128, S]
        sinb = np.concatenate([-s, s], axis=0)
        in_maps.append({
            "xT": xT_b, "wq": wq_r, "wk": wk_r, "wv": wv_r, "wo": wo_r,
            "cosb": np.ascontiguousarray(cosb),
            "sinb": np.ascontiguousarray(sinb),
        })
    return in_maps


def kernel(x, freqs_cos, freqs_sin, wq, wk, wv, wo):
    from concourse.bass_utils import run_bass_kernel_spmd

    if "nc" not in _CACHE:
        _CACHE["nc"] = _build()
    nc = _CACHE["nc"]

    in_maps = make_in_maps(x, freqs_cos, freqs_sin, wq, wk, wv, wo)

    res = run_bass_kernel_spmd(nc, in_maps, core_ids=list(range(B)))
    _CACHE["last_results"] = res
    outs = [r["out"].reshape(S, DIM) for r in res.results]
    return np.concatenate(outs, axis=0)


# revision 28
# speedup vs baseline: 68.4716x; 1.0701x over previous
"""Trainium2 Bass kernel for nn_Attention_9354438771128.

GQA attention block (Mistral-style): QKV projections + RoPE + block-diagonal
(8 x 1024) full attention + output projection, fp32 reference.

Sharding: data-parallel over the 8 sequence blocks, one block per NeuronCore.
Each core computes its block's full attention independently (no collectives).

Per-core pipeline (all matmuls bf16 with fp32 PSUM accumulation):
  - host pre-work: x^T slices, per-head even/odd column permutation of wq/wk
    (turns interleaved RoPE into a half-rotation), RoPE cos/sin tables in
    transposed layout, wv/wo pre-tiled so every weight DMA is one fully
    contiguous block.
  - k^T/q^T computed per head directly in [head_dim, seq] layout; RoPE applied
    with 4 DVE ops using partition-offset operands (no SBUF->SBUF swap DMA).
  - per head: scores^T = k^T.T @ q^T into a 2-bank [128,1024] PSUM tile
    (LDW shared between the two 512-wide matmuls); exp on ScalarE (fused
    scale, no max subtraction -- scores bounded ~|9|); running softmax
    denominator on DVE; PV accumulates A^T = V^T P^T in PSUM; denominator
    all-reduced+broadcast across partitions by GpSimd partition_all_reduce;
    normalization fused into the PSUM evacuation; A^T staged to DRAM (bf16,
    one contiguous store per head).
  - out = A @ wo streamed per 512-column block, one contiguous DMA per block.
"""

import sys

sys.path.insert(0, "/opt/trn_rl_repo")

import numpy as np
import ml_dtypes

BF = ml_dtypes.bfloat16

B, S, DIM = 8, 1024, 4096
NH, NKV, HD = 32, 8, 128
KC = DIM // 128            # 32 contraction chunks
TT = S // 128              # 8 token tiles per block
SCALE = HD ** -0.5

_CACHE = {}


def _build(repeat=1, phases="BDE", loop=0, pipeline_heads=True,
           early_evac=False, psum_alt=False, nogp=False, bf16_rope=False,
           ops_bufs=6, wo_first=False, dma_split=False):
    import concourse.bass as bass
    import concourse.mybir as mybir
    from concourse import bacc, bass_utils
    from concourse.tile import TileContext
    from bass_rust import ReduceOp

    # let walrus elide back-to-back identical weight loads
    if not getattr(bass_utils.get_walrus_args, "_ldw_opt", False):
        _orig = bass_utils.get_walrus_args

        def _patched(*a, **k):
            return [x.replace("--enable-ldw-opt=false", "--enable-ldw-opt=true")
                    for x in _orig(*a, **k)]

        _patched._ldw_opt = True
        bass_utils.get_walrus_args = _patched

    f32 = mybir.dt.float32
    bf16 = mybir.dt.bfloat16
    Exp = mybir.ActivationFunctionType.Exp
    mult = mybir.AluOpType.mult
    add = mybir.AluOpType.add

    nc = bacc.Bacc("TRN2", num_devices=8)

    xT = nc.dram_tensor("xT", [KC, 128, S], bf16, kind="ExternalInput")
    wq = nc.dram_tensor("wq", [NH, 128, DIM], bf16, kind="ExternalInput")
    wk = nc.dram_tensor("wk", [NKV, 128, DIM], bf16, kind="ExternalInput")
    wv = nc.dram_tensor("wv", [2, 128, KC, 512], bf16, kind="ExternalInput")
    wo = nc.dram_tensor("wo", [8, 128, NH, 512], bf16, kind="ExternalInput")
    cosb = nc.dram_tensor("cosb", [128, S], f32, kind="ExternalInput")
    sinb = nc.dram_tensor("sinb", [128, S], f32, kind="ExternalInput")
    out = nc.dram_tensor("out", [TT, 128, DIM], f32, kind="ExternalOutput")
    at_dram = nc.dram_tensor("at_scratch", [NH, 128, S], bf16, kind="Internal")

    def body(tc):
        with tc.tile_pool(name="const", bufs=1) as cpool:
            cos_t = cpool.tile([128, S], f32)
            sin_t = cpool.tile([128, S], f32)
            nc.sync.dma_start(cos_t, cosb[:])
            nc.sync.dma_start(sin_t, sinb[:])
            if nogp:
                ones_col = cpool.tile([128, 1], bf16)
                ones_row = cpool.tile([1, 128], bf16)
                nc.vector.memset(ones_col, 1.0)
                nc.vector.memset(ones_row, 1.0)
            if bf16_rope:
                cos_tb = cpool.tile([128, S], bf16)
                sin_tb = cpool.tile([128, S], bf16)
                with nc.allow_low_precision(reason="bf16 rope tables"):
                    nc.vector.tensor_copy(cos_tb, cos_t)
                    nc.vector.tensor_copy(sin_tb, sin_t)

            def rope_store(psum_half, dst, sl):
                # dst = psum * cos + halfswap(psum) * sin  (sign folded into
                # the host-built sin table); partition-offset DVE operands
                # replace the SBUF->SBUF partition-swap DMA.
                if bf16_rope:
                    # ACT evacuates PSUM to bf16 so all 4 DVE ops run in the
                    # 2x 16-bit mode (probe: shifts ~2.8us/head DVE -> ACT)
                    raw = rpool.tile([128, 512], bf16, tag="rope_raw")
                    with nc.allow_low_precision(reason="bf16 rope"):
                        nc.scalar.copy(raw, psum_half)
                        t1 = rpool.tile([128, 512], bf16, tag="rope_t1")
                        t2 = rpool.tile([128, 512], bf16, tag="rope_t2")
                        nc.vector.tensor_tensor(t1, raw, cos_tb[:, sl], mult)
                        nc.vector.tensor_tensor(
                            t2[0:64], raw[64:128], sin_tb[0:64, sl], mult)
                        nc.vector.tensor_tensor(
                            t2[64:128], raw[0:64], sin_tb[64:128, sl], mult)
                        nc.vector.tensor_tensor(dst, t1, t2, add)
                    return
                t1 = rpool.tile([128, 512], f32, tag="rope_t1")
                t2 = rpool.tile([128, 512], f32, tag="rope_t2")
                nc.vector.tensor_tensor(t1, psum_half, cos_t[:, sl], mult)
                nc.vector.tensor_tensor(
                    t2[0:64], psum_half[64:128], sin_t[0:64, sl], mult)
                nc.vector.tensor_tensor(
                    t2[64:128], psum_half[0:64], sin_t[64:128, sl], mult)
                nc.vector.tensor_tensor(dst, t1, t2, add)

            with tc.tile_pool(name="xt", bufs=1) as xtpool, \
                 tc.tile_pool(name="wstream", bufs=3) as wpool, \
                 tc.tile_pool(name="rope", bufs=2) as rpool, \
                 tc.tile_pool(name="kv", bufs=1) as kvpool, \
                 tc.tile_pool(name="qkps", bufs=2, space="PSUM") as qkps:
                xt_t = xtpool.tile([128, KC, S], bf16)
                for kc in range(KC):
                    nc.sync.dma_start(xt_t[:, kc], xT[kc])

                kt_t = kvpool.tile([128, NKV, S], bf16)
                v_t = kvpool.tile([128, TT, NKV * HD], bf16)

                # ---------------- Phase B: K^T (roped) and V ----------------
                for g in range(NKV):
                    wk_t = wpool.tile([128, DIM], bf16, tag="wqk")
                    nc.sync.dma_start(wk_t, wk[g])
                    for ch in range(2):
                        sl = slice(ch * 512, (ch + 1) * 512)
                        ps = qkps.tile([128, 512], f32, tag="qk")
                        for kc in range(KC):
                            nc.tensor.matmul(
                                ps, wk_t[:, kc * 128:(kc + 1) * 128],
                                xt_t[:, kc, sl],
                                start=(kc == 0), stop=(kc == KC - 1))
                        rope_store(ps, kt_t[:, g, sl], sl)

                with tc.tile_pool(name="wvstream", bufs=2) as wvpool, \
                     tc.tile_pool(name="vps", bufs=2, space="PSUM") as vps:
                    for vc in range(2):
                        wv_t = wvpool.tile([128, KC, 512], bf16)
                        nc.sync.dma_start(wv_t, wv[vc])
                        for tt in range(TT):
                            ps = vps.tile([128, 512], f32)
                            for kc in range(KC):
                                nc.tensor.matmul(
                                    ps, xt_t[:, kc, tt * 128:(tt + 1) * 128],
                                    wv_t[:, kc],
                                    start=(kc == 0), stop=(kc == KC - 1))
                            nc.vector.tensor_copy(
                                v_t[:, tt, vc * 512:(vc + 1) * 512], ps)

                # ---------------- Phase D: per-head Q + attention ----------------
                if "D" not in phases:
                    nc.gpsimd.dma_start(
                        out[0, :, :S],
                        kt_t.rearrange("p a b -> p (a b)")[:, :S])
                    nc.gpsimd.dma_start(
                        out[1, :, :S],
                        v_t.rearrange("p a b -> p (a b)")[:, :S])
                    return
                with tc.tile_pool(name="qt", bufs=3) as qtpool, \
                     tc.tile_pool(name="expt", bufs=4) as epool, \
                     tc.tile_pool(name="esum", bufs=4) as espool, \
                     tc.tile_pool(name="nrm", bufs=2) as npool, \
                     tc.tile_pool(name="atst", bufs=2) as atpool, \
                     tc.tile_pool(name="sps", bufs=(1 if psum_alt else 2),
                                  space="PSUM") as sps, \
                     tc.tile_pool(name="aps", bufs=(2 if psum_alt else 1),
                                  space="PSUM") as aps:

                    def qproj(h, qt_t):
                        wq_t = wpool.tile([128, DIM], bf16, tag="wqk")
                        nc.sync.dma_start(wq_t, wq[h])
                        for ch in range(2):
                            sl = slice(ch * 512, (ch + 1) * 512)
                            ps = qkps.tile([128, 512], f32, tag="qk")
                            for kc in range(KC):
                                nc.tensor.matmul(
                                    ps, wq_t[:, kc * 128:(kc + 1) * 128],
                                    xt_t[:, kc, sl],
                                    start=(kc == 0), stop=(kc == KC - 1))
                            rope_store(ps, qt_t[:, sl], sl)

                    qts = {}
                    if pipeline_heads:
                        qts[0] = qtpool.tile([128, S], bf16, tag="qt_t", name="qt_t")
                        qproj(0, qts[0])
                    for h in range(NH):
                        g = h // 4
                        if pipeline_heads:
                            if h + 1 < NH:
                                qts[h + 1] = qtpool.tile(
                                    [128, S], bf16, tag="qt_t", name="qt_t")
                                qproj(h + 1, qts[h + 1])
                            qt_t = qts.pop(h)
                        else:
                            qt_t = qtpool.tile([128, S], bf16, tag="qt_t")
                            qproj(h, qt_t)

                        a_ps = aps.tile([128, S], f32)
                        es_prev = None
                        for sk in range(TT):
                            kt_sl = kt_t[:, g, sk * 128:(sk + 1) * 128]
                            s_ps = sps.tile([128, S], f32)
                            nc.tensor.matmul(
                                s_ps[:, 0:512], kt_sl, qt_t[:, 0:512],
                                start=True, stop=True)
                            nc.tensor.matmul(
                                s_ps[:, 512:1024], kt_sl, qt_t[:, 512:1024],
                                start=True, stop=True)
                            e_t = epool.tile([128, S], bf16)
                            nc.scalar.activation(e_t, s_ps, Exp, scale=SCALE)
                            if es_prev is None:
                                es_prev = e_t
                            else:
                                es_new = espool.tile([128, S], bf16, tag="es")
                                nc.vector.tensor_tensor(es_new, es_prev, e_t, add)
                                es_prev = es_new
                            v_sl = v_t[:, sk, g * 128:(g + 1) * 128]
                            nc.tensor.matmul(
                                a_ps[:, 0:512], v_sl, e_t[:, 0:512],
                                start=(sk == 0), stop=(sk == TT - 1))
                            nc.tensor.matmul(
                                a_ps[:, 512:1024], v_sl, e_t[:, 512:1024],
                                start=(sk == 0), stop=(sk == TT - 1))

                        if nogp:
                            # denominator without GpSimd (its SBUF port lock
                            # stalls DVE): PE ones-matmul partition reduction,
                            # DVE reciprocal, PE outer-product broadcast,
                            # ScalarE copy to SBUF for the DVE normalize.
                            n_ps0 = qkps.tile([1, 512], f32, tag="qk",
                                              name="n_ps0")
                            n_ps1 = qkps.tile([1, 512], f32, tag="qk",
                                              name="n_ps1")
                            nc.tensor.matmul(n_ps0, ones_col,
                                             es_prev[:, 0:512],
                                             start=True, stop=True)
                            nc.tensor.matmul(n_ps1, ones_col,
                                             es_prev[:, 512:1024],
                                             start=True, stop=True)
                            rec1_t = npool.tile([1, S], bf16, tag="rec1")
                            with nc.allow_low_precision(
                                    reason="bf16 reciprocal feeds bf16 matmul"):
                                nc.vector.reciprocal(rec1_t[:, 0:512], n_ps0)
                                nc.vector.reciprocal(
                                    rec1_t[:, 512:1024], n_ps1)
                            nb_ps = sps.tile([128, S], f32, tag="s_ps",
                                             name="nb_ps")
                            nc.tensor.matmul(nb_ps[:, 0:512], ones_row,
                                             rec1_t[:, 0:512],
                                             start=True, stop=True)
                            nc.tensor.matmul(nb_ps[:, 512:1024], ones_row,
                                             rec1_t[:, 512:1024],
                                             start=True, stop=True)
                            rec_t = npool.tile([128, S], f32, tag="nb")
                            nc.scalar.copy(rec_t, nb_ps)
                        else:
                            nb_t = npool.tile([128, S], f32, tag="nb")
                            nc.gpsimd.partition_all_reduce(
                                nb_t, es_prev, 128, ReduceOp.add)
                            rec_t = npool.tile([128, S], f32, tag="rec")
                            nc.vector.reciprocal(rec_t, nb_t)
                        at_t = atpool.tile([128, S], bf16)
                        if early_evac:
                            # free the PSUM bank before the gpsimd/recip chain
                            # completes so the next head's PV is not gated on it
                            ar_t = npool.tile([128, S], f32, tag="ar")
                            nc.vector.tensor_copy(ar_t, a_ps)
                            nc.vector.tensor_tensor(at_t, ar_t, rec_t, mult)
                        else:
                            nc.vector.tensor_tensor(at_t, a_ps, rec_t, mult)
                        if dma_split:
                            nc.scalar.dma_start(at_dram[h], at_t)
                        else:
                            nc.sync.dma_start(at_dram[h], at_t)

        # ---------------- Phase E: out = A @ wo ----------------
        if "E" not in phases:
            return
        with tc.tile_pool(name="atrd", bufs=1) as atrd, \
             tc.tile_pool(name="wo", bufs=3) as wopool, \
             tc.tile_pool(name="ost", bufs=6) as opool, \
             tc.tile_pool(name="ops", bufs=ops_bufs, space="PSUM") as ops:
            at_all = atrd.tile([128, NH, S], bf16)
            wo_pre = []
            if wo_first:
                wo_t0 = wopool.tile([128, NH, 512], bf16, tag="wo_t")
                nc.sync.dma_start(wo_t0, wo[0])
                wo_pre.append(wo_t0)
            for h in range(NH):
                if dma_split:
                    nc.scalar.dma_start(at_all[:, h], at_dram[h])
                else:
                    nc.sync.dma_start(at_all[:, h], at_dram[h])
            for nch in range(8):
                nsl = slice(nch * 512, (nch + 1) * 512)
                if nch < len(wo_pre):
                    wo_t = wo_pre[nch]
                else:
                    wo_t = wopool.tile([128, NH, 512], bf16, tag="wo_t")
                    nc.sync.dma_start(wo_t, wo[nch])
                for tt in range(TT):
                    o_ps = ops.tile([128, 512], f32)
                    for h in range(NH):
                        nc.tensor.matmul(
                            o_ps, at_all[:, h, tt * 128:(tt + 1) * 128],
                            wo_t[:, h],
                            start=(h == 0), stop=(h == NH - 1))
                    o_t = opool.tile([128, 512], f32)
                    nc.vector.tensor_copy(o_t, o_ps)
                    nc.sync.dma_start(out[tt, :, nsl], o_t)

    with TileContext(nc) as tc:
        if loop > 1:
            with tc.For_i(0, loop, 1):
                body(tc)
        else:
            for _rep in range(repeat):
                body(tc)

    nc.compile()
    return nc


def _prep_shared(wq, wk, wv, wo):
    idx = np.arange(128)
    ph = np.concatenate([idx[0::2], idx[1::2]])
    permq = (np.arange(NH)[:, None] * HD + ph[None, :]).reshape(-1)
    permk = (np.arange(NKV)[:, None] * HD + ph[None, :]).reshape(-1)
    wq_r = np.ascontiguousarray(
        wq[:, permq].reshape(KC, 128, NH, HD).transpose(2, 1, 0, 3)
    ).reshape(NH, 128, DIM).astype(BF)
    wk_r = np.ascontiguousarray(
        wk[:, permk].reshape(KC, 128, NKV, HD).transpose(2, 1, 0, 3)
    ).reshape(NKV, 128, DIM).astype(BF)
    wv_r = np.ascontiguousarray(
        wv.reshape(KC, 128, 2, 512).transpose(2, 1, 0, 3)).astype(BF)
    wo_r = np.ascontiguousarray(
        wo.reshape(NH, 128, 8, 512).transpose(2, 1, 0, 3)).astype(BF)
    return wq_r, wk_r, wv_r, wo_r


def make_in_maps(x, freqs_cos, freqs_sin, wq, wk, wv, wo):
    wq_r, wk_r, wv_r, wo_r = _prep_shared(
        np.asarray(wq, np.float32), np.asarray(wk, np.float32),
        np.asarray(wv, np.float32), np.asarray(wo, np.float32))

    x = np.asarray(x, np.float32)
    fc = np.asarray(freqs_cos, np.float32)
    fs = np.asarray(freqs_sin, np.float32)

    in_maps = []
    for b in range(B):
        xb = x[b * S:(b + 1) * S]                       # [S, DIM]
        xT_b = np.ascontiguousarray(xb.T).astype(BF).reshape(KC, 128, S)
        c = np.ascontiguousarray(fc[b * S:(b + 1) * S].T.astype(np.float32))
        s = np.ascontiguousarray(fs[b * S:(b + 1) * S].T.astype(np.float32))
        cosb = np.concatenate([c, c], axis=0)           # [128, S]
        sinb = np.concatenate([-s, s], axis=0)
        in_maps.append({
            "xT": xT_b, "wq": wq_r, "wk": wk_r, "wv": wv_r, "wo": wo_r,
            "cosb": np.ascontiguousarray(cosb),
            "sinb": np.ascontiguousarray(sinb),
        })
    return in_maps


def kernel(x, freqs_cos, freqs_sin, wq, wk, wv, wo):
    from concourse.bass_utils import run_bass_kernel_spmd

    if "nc" not in _CACHE:
        _CACHE["nc"] = _build()
    nc = _CACHE["nc"]

    in_maps = make_in_maps(x, freqs_cos, freqs_sin, wq, wk, wv, wo)

    res = run_bass_kernel_spmd(nc, in_maps, core_ids=list(range(B)))
    _CACHE["last_results"] = res
    outs = [r["out"].reshape(S, DIM) for r in res.results]
    return np.concatenate(outs, axis=0)
